# revision 24
# baseline (speedup 1.0000x reference)
"""MoE layer (top-k routing) on 8 Trainium2 NeuronCores.

Expert-parallel per the sharding hint: the host computes router softmax +
top-k (0.1% of FLOPs) and realizes the "all-to-all dispatch by expert
assignment" while building the per-core SPMD input maps; each core runs
expert FFN work in bf16 (fp32 PSUM accumulation); the host applies the
combine weights and scatter-adds results back to [B,N,C].

Load balance: each expert's FFN is split along D_FF into four quarter-units
(exact: gelu is elementwise over F and GEMM2 contracts F, so the four
partial y's just add). The 32 quarter-units are assigned four per core, one
per slot class A-D: slot A holds the two largest experts' quarters, slot B
the next two, etc. Each slot is padded to the max count within its pair, so
per-core padded work is sum over slots of max(pair) — within ~1% of the
perfect-balance floor — instead of 2*max(all counts).
"""

import json
import os
import sys
import types

import numpy as np
import ml_dtypes

D_MODEL = 1024
D_FF = 4096
N_EXPERTS = 8
N_CORES = 8

P = 128
CB = D_MODEL // P      # 8 c-blocks of 128
FQ = D_FF // 4         # F quarter = 1024
FBQ = FQ // P          # 8 f-blocks per quarter
TN = 512               # token tile (matmul moving free dim / one PSUM bank)
SLOTS = ("A", "B", "C", "D")


def _shim_axon_hooks():
    """Register the NTFF profile hook bass_utils looks for under axon; the
    image's `antenv` stub lacks `axon_hooks`."""
    if "antenv.axon_hooks" in sys.modules:
        return
    try:
        import trn_agent_boot.trn_boot as _tb
        hook = _tb._ntff_profile_via_ctypes("/opt/axon/libaxon_pjrt.so")
    except Exception:
        hook = None
    mod = types.ModuleType("antenv.axon_hooks")
    mod.get_axon_ntff_profile_hook = lambda: hook
    mod.set_axon_ntff_profile_hook = lambda h: None
    sys.modules["antenv.axon_hooks"] = mod


_shim_axon_hooks()

import concourse.bass as bass            # noqa: E402
import concourse.tile as tile            # noqa: E402
from concourse import mybir              # noqa: E402
from concourse.bass import ds, ts        # noqa: E402
from concourse.bass_utils import run_bass_kernel_spmd  # noqa: E402


def _fix_multiwait_bir(nc):
    """Split instructions carrying >1 sync wait (the TileContext tail drain)
    into single-wait NoOps; this walrus build rejects multi-wait CTRL
    instructions."""
    raw = bass.Bass.to_json_bytes(nc)
    d = json.loads(raw)
    for f in d["functions"]:
        for b in f["blocks"]:
            out = []
            for i in b["instructions"]:
                si = i.get("sync_info") or {}
                waits = si.get("on_wait") or []
                if len(waits) > 1:
                    for k, w in enumerate(waits[:-1]):
                        out.append({
                            "name": f"{i['name']}_wsplit{k}",
                            "engine": i["engine"],
                            "ins": [], "outs": [],
                            "opcode": "NoOp",
                            "sync_info": {"on_update": [], "on_wait": [w]},
                        })
                    si["on_wait"] = [waits[-1]]
                out.append(i)
            b["instructions"] = out
    fixed = json.dumps(d).encode()
    nc.to_json_bytes = lambda: fixed


_NC_CACHE = {}


def _token_tiles(cap, small_first=False):
    # small_first: a 256-token leading tile halves the bytes the very first
    # matmul waits on; later tiles are prefetched early enough to cover it
    tiles, off = [], 0
    if small_first and cap > TN:
        tiles.append((0, 256))
        off = 256
    while off < cap:
        tw = min(TN, cap - off)
        tiles.append((off, tw))
        off += tw
    return tiles


def _build_moe_kernel(caps):
    """Four quarter-expert FFN units per core (slots A-D), SPMD x8."""
    key = tuple(caps)
    if key in _NC_CACHE:
        return _NC_CACHE[key]

    bf16 = mybir.dt.bfloat16
    f32 = mybir.dt.float32
    Act = mybir.ActivationFunctionType

    nc = bass.Bass("TRN2", target_bir_lowering=False, debug=False,
                   num_devices=N_CORES)

    # all 8 per-slot bias vectors pre-rearranged by the host into one
    # [128, 64] blob: one 128-descriptor DMA instead of 8 DMAs of 1024
    # 4-byte descriptors each (~0.8us queue time + heavy DMA-engine load)
    biasP = nc.declare_dram_parameter("bias", [P, 4 * (FBQ + CB)], f32,
                                      isOutput=False)
    # All DRAM tensors are host-repacked to partition-major layouts so each
    # DMA descriptor covers a full per-partition line (8-16KB) instead of a
    # 1-2KB row segment: the DMA system is descriptor-rate-bound (~80ns per
    # descriptor per engine), not byte-bound, so fat descriptors are what
    # make loads land fast.
    units = []
    for si, (slot, cap) in enumerate(zip(SLOTS, caps)):
        u = {"cap": cap, "slot": slot, "boff": si * (FBQ + CB)}
        u["tiles"] = _token_tiles(cap)
        nt = len(u["tiles"])
        u["nt"] = nt
        # x/y chunk-major: [128, ntiles, 8, 512] -> per-(p,tile) 8KB runs
        u["xT"] = nc.declare_dram_parameter(f"xT{slot}", [P, nt, CB, TN], bf16, isOutput=False)
        # w: [128, 8, 1024] -> per-p 16KB runs
        u["w1t"] = nc.declare_dram_parameter(f"w1t{slot}", [P, CB, FQ], bf16, isOutput=False)
        u["w2t"] = nc.declare_dram_parameter(f"w2t{slot}", [P, FBQ, D_MODEL], bf16, isOutput=False)
        # partials return as bf16: halves the output DMA so total traffic
        # stays under the chip's P0 power-throttle trigger (observed: the
        # f32 version pushed PE from 2.4 to 2.0 GHz); host sums in f32
        u["yT"] = nc.declare_dram_parameter(f"yT{slot}", [P, nt, CB, TN], bf16, isOutput=True)
        u["xr"] = u["xT"].ap()      # [128, nt, 8, 512]
        u["w1r"] = u["w1t"].ap()    # [128, 8, 1024]
        u["w2r"] = u["w2t"].ap()    # [128, 8, 1024]
        u["yr"] = u["yT"].ap()
        units.append(u)

    with tile.TileContext(nc) as tc:
        with (
            tc.tile_pool(name="weights", bufs=1) as wpool,
            tc.tile_pool(name="xin", bufs=3) as xpool,
            tc.tile_pool(name="hbuf", bufs=1) as hpool,
            tc.tile_pool(name="yout", bufs=3) as ypool,
            tc.tile_pool(name="psum", bufs=4, space="PSUM") as psum,
            tc.tile_pool(name="warm", bufs=1) as warmpool,
        ):
            # PE clock warm-up: ~20 dependency-free matmuls on a memset tile
            # run while the first loads are in flight. A cold PE runs its
            # first ~15 matmuls at half clock (427-609ns vs 215ns); burning
            # the ramp on dummies makes the real tile-0 matmuls full-speed.
            wsrc = warmpool.tile([P, P + TN], bf16, tag="wsrc", name="wsrc")
            nc.vector.memset(wsrc[:, :], 0)
            # borrow a rotation slot from the main psum pool's ph tag; the
            # warm matmuls retire long before the 4-deep rotation reuses it
            wps = psum.tile([P, TN], f32, tag="ph", name="wps")
            for _ in range(20):
                nc.tensor.matmul(wps[:, :], lhsT=wsrc[:, 0:P],
                                 rhs=wsrc[:, P:P + TN], start=True, stop=True)
            # ---- loads. SP-ring FIFO order is chosen so PE never waits:
            # tile-0 tokens + first w1 strip first (w1 strip on the ACT ring
            # so it overlaps x0's load), then unit A's remaining weights,
            # then unit A's tile-1 tokens BEFORE units B-D's weight bulk so
            # early tiles stay ahead of the PE.
            # chunked so the first matmul waits on 128KB, not a 1MB monolith:
            # x0 per-k on the sync ring; w1A/w2A chunked along the contract
            # group g (keeps 2KB descriptors — chunking along f/c would 8x
            # the descriptor count). w1A is split across the gpsimd and
            # scalar rings so G1(tile0) is fed at ~2 rings' bandwidth. The
            # scalar queue gets only 4 early dma_starts (issued well before
            # the first gelu needs the engine).
            ua = units[0]
            ua["x0"] = xpool.tile([P, CB, TN], bf16, tag="xt", name="x0A")
            ua["w1_sb"] = wpool.tile([P, CB, FQ], bf16, tag="w1A", name="w1A")
            # measured queue behavior: sync moves fat descriptors fast
            # (~280GB/s at 8KB), scalar does ~160GB/s with 4 small chunks
            # outstanding, gpsimd's ring starts ~2us late. So: first-needed
            # w1 chunks as 4x 2KB-desc dma_starts on scalar, x0 + the w1
            # back half as fat loads on sync, everything else on gpsimd.
            for g in range(4):
                nc.scalar.dma_start(ua["w1_sb"][:, g, :], ua["w1r"][:, g, :])
            nc.sync.dma_start(ua["x0"][:, :, :], ua["xr"][:, 0, :, :])
            nc.sync.dma_start(ua["w1_sb"][:, 4:CB, :], ua["w1r"][:, 4:CB, :])
            bias_sb = wpool.tile([P, 4 * (FBQ + CB)], f32, tag="bias",
                                 name="bias")
            nc.gpsimd.dma_start(bias_sb[:, :], biasP.ap())
            ua["w2_sb"] = wpool.tile([P, FBQ, D_MODEL], bf16, tag="w2A", name="w2A")
            nc.gpsimd.dma_start(ua["w2_sb"][:, 0:4, :], ua["w2r"][:, 0:4, :])
            nc.gpsimd.dma_start(ua["w2_sb"][:, 4:FBQ, :], ua["w2r"][:, 4:FBQ, :])

            # unit A tiles 1 and 2 ahead of the remaining weight bulk (the
            # tile-2 load sat behind 12MB of B-D weights on the sync ring and
            # cost an 8.8us PE stall)
            for ti in (1, 2):
                if len(ua["tiles"]) > ti:
                    xt = xpool.tile([P, CB, TN], bf16, tag="xt", name=f"x{ti}A")
                    nc.sync.dma_start(xt[:, :, :], ua["xr"][:, ti, :, :])
                    ua[f"x{ti}"] = xt

            # B-D weight bulk split across the sync and gpsimd rings so it
            # drains ~2x faster and the sync ring frees up for x/y tiles
            for wi, u in enumerate(units[1:]):
                slot = u["slot"]
                u["w1_sb"] = wpool.tile([P, CB, FQ], bf16, tag=f"w1{slot}",
                                        name=f"w1{slot}")
                u["w2_sb"] = wpool.tile([P, FBQ, D_MODEL], bf16, tag=f"w2{slot}",
                                        name=f"w2{slot}")
            for u in units[1:]:
                nc.sync.dma_start(u["w1_sb"][:, :, :], u["w1r"][:, :, :])
                nc.gpsimd.dma_start(u["w2_sb"][:, :, :], u["w2r"][:, :, :])

            # ---- compute: unit A..D tiles in sequence ----
            for u in units:
                for ti, (off, tw) in enumerate(u["tiles"]):
                    if f"x{ti}" in u:
                        xt = u[f"x{ti}"]
                    else:
                        xt = xpool.tile([P, CB, TN], bf16, tag="xt")
                        nc.sync.dma_start(xt[:, :, :], u["xr"][:, ti, :, :])

                    ht = hpool.tile([P, FBQ, TN], bf16, tag="ht")
                    for m in range(FBQ):
                        ph = psum.tile([P, TN], f32, tag="ph")
                        for k in range(CB):
                            nc.tensor.matmul(
                                ph[:, :tw],
                                lhsT=u["w1_sb"][:, k, ts(m, P)],
                                rhs=xt[:, k, :tw],
                                start=(k == 0), stop=(k == CB - 1),
                            )
                        nc.scalar.activation(ht[:, m, :tw], ph[:, :tw], Act.Gelu,
                                             bias=bias_sb[:, u["boff"] + m:u["boff"] + m + 1])

                    last = (u is units[-1]) and (ti == len(u["tiles"]) - 1)
                    yt = ypool.tile([P, CB, TN], bf16, tag="yt")
                    for c in range(CB):
                        py = psum.tile([P, TN], f32, tag="py")
                        for k in range(FBQ):
                            nc.tensor.matmul(
                                py[:, :tw],
                                lhsT=u["w2_sb"][:, k, ts(c, P)],
                                rhs=ht[:, k, :tw],
                                start=(k == 0), stop=(k == FBQ - 1),
                            )
                        bo = u["boff"] + FBQ
                        nc.scalar.add(yt[:, c, :tw], py[:, :tw],
                                      bias_sb[:, bo + c:bo + c + 1])
                        if last:
                            # final tile: per-block stores overlap the tail
                            # GEMM2 blocks instead of one post-loop DMA
                            nc.sync.dma_start(u["yr"][:, ti, c, :tw],
                                              yt[:, c, :tw])
                    if not last:
                        # full-chunk store (8KB descriptors); the pad zone
                        # carries stale data the host never reads
                        nc.sync.dma_start(u["yr"][:, ti, :, :], yt[:, :, :])

    _fix_multiwait_bir(nc)
    _NC_CACHE[key] = nc
    return nc


def _route(xf, router_w, k):
    """Replicate the reference router numerics (f32 softmax, top-k, renorm)."""
    logits = xf @ router_w.T.astype(np.float32)          # [T, E]
    m = logits.max(axis=-1, keepdims=True)
    e = np.exp(logits - m, dtype=np.float32)
    probs = e / e.sum(axis=-1, keepdims=True)
    # descending, ties -> lower index first (matches jax.lax.top_k)
    idx = np.argsort(-probs, axis=-1, kind="stable")[:, :k]   # [T, k]
    w = np.take_along_axis(probs, idx, axis=-1)               # [T, k]
    w = w / (w.sum(axis=-1, keepdims=True) + 1e-9)
    return idx, w


def _align16(n):
    # 4-token (8-byte) alignment keeps DMA rows aligned; finer than 16
    # saves ~20 padded tokens across the four slots
    return max(P, -(-n // 4) * 4)


def kernel(x, router_w, expert_w1, expert_b1, expert_w2, expert_b2, top_k):
    x = np.asarray(x)
    router_w = np.asarray(router_w, dtype=np.float32)
    expert_w1 = np.asarray(expert_w1, dtype=np.float32)
    expert_b1 = np.asarray(expert_b1, dtype=np.float32)
    expert_w2 = np.asarray(expert_w2, dtype=np.float32)
    expert_b2 = np.asarray(expert_b2, dtype=np.float32)
    k = int(np.asarray(top_k))
    Bq, Nq, C = x.shape
    Tq = Bq * Nq
    E = expert_w1.shape[0]
    xf = np.ascontiguousarray(x.reshape(Tq, C), dtype=np.float32)

    idx, w = _route(xf, router_w, k)

    tok_idx, tok_w = [], []
    for e in range(E):
        mask = idx == e
        sel = np.nonzero(mask.any(axis=-1))[0]
        tok_idx.append(sel)
        tok_w.append((w * mask).sum(axis=-1)[sel].astype(np.float32))
    counts = np.array([len(s) for s in tok_idx])

    # slot s holds the quarters of the experts ranked 2s and 2s+1 by count;
    # cores 0-3 take quarters 0-3 of the first, cores 4-7 of the second
    order = np.argsort(-counts, kind="stable")
    caps = [_align16(int(counts[order[2 * s]])) for s in range(4)]

    nc = _build_moe_kernel(caps)

    # one xT per expert, shared by its four quarter-units; chunk-major
    # [128, ntiles, 8, 512] so every tile load is 128 8KB descriptors
    xTs, slot_of = {}, {}
    for s in range(4):
        nt = (caps[s] + TN - 1) // TN
        for j in (0, 1):
            e = int(order[2 * s + j])
            slot_of[e] = s
            tmp = np.zeros((nt * TN, C), dtype=np.float32)
            tmp[:counts[e]] = xf[tok_idx[e]]
            xTs[e] = np.ascontiguousarray(
                tmp.reshape(nt, TN, CB, P).transpose(3, 0, 2, 1)
            ).astype(ml_dtypes.bfloat16)

    in_maps = [dict() for _ in range(N_CORES)]
    placement = {}          # (expert, quarter) -> (core, slot name)
    FBQ_, CB_ = FQ // P, C // P
    blobs = [np.zeros((P, 4 * (FBQ_ + CB_)), dtype=np.float32)
             for _ in range(N_CORES)]
    for s, slot in enumerate(SLOTS):
        for core in range(N_CORES):
            e = int(order[2 * s + (0 if core < 4 else 1)])
            q = core % 4
            placement[(e, q)] = (core, slot)
            f0, f1 = q * FQ, (q + 1) * FQ
            b2 = expert_b2[e] if q == 0 else np.zeros(C, dtype=np.float32)
            bo = s * (FBQ_ + CB_)
            blobs[core][:, bo:bo + FBQ_] = expert_b1[e, f0:f1].reshape(FBQ_, P).T
            blobs[core][:, bo + FBQ_:bo + FBQ_ + CB_] = b2.reshape(CB_, P).T
            w1q = expert_w1[e, f0:f1].T.reshape(CB_, P, FQ).transpose(1, 0, 2)
            w2q = expert_w2[e, :, f0:f1].T.reshape(FBQ_, P, C).transpose(1, 0, 2)
            in_maps[core].update({
                f"xT{slot}": xTs[e],
                f"w1t{slot}": np.ascontiguousarray(w1q).astype(ml_dtypes.bfloat16),
                f"w2t{slot}": np.ascontiguousarray(w2q).astype(ml_dtypes.bfloat16),
            })
    for core in range(N_CORES):
        in_maps[core]["bias"] = blobs[core]

    trace = os.environ.get("BASS_MOE_TRACE") == "1"
    res = run_bass_kernel_spmd(
        nc, in_maps, core_ids=list(range(N_CORES)),
        trace=trace,
        tmpdir=os.environ.get("BASS_MOE_TMPDIR") if trace else None,
    )
    if trace:
        kernel.last_exec_time_ns = res.exec_time_ns
        kernel.last_trace = (res.instructions_and_trace or (None, None))[1]

    out = np.zeros((Tq, C), dtype=np.float32)
    for e in range(E):
        cnt = counts[e]
        if not cnt:
            continue
        acc = np.zeros((cnt, C), dtype=np.float32)
        for q in range(4):
            core, slot = placement[(e, q)]
            yq = res.results[core][f"yT{slot}"]        # [128, nt, 8, 512]
            nt = yq.shape[1]
            yq = yq.transpose(1, 3, 2, 0).reshape(nt * TN, C)[:cnt]
            acc += yq.astype(np.float32)
        out[tok_idx[e]] += acc * tok_w[e][:, None]
    return out.reshape(Bq, Nq, C).astype(x.dtype)



# revision 25
# speedup vs baseline: 1.0229x; 1.0229x over previous
"""MoE layer (top-k routing) on 8 Trainium2 NeuronCores.

Expert-parallel per the sharding hint: the host computes router softmax +
top-k (0.1% of FLOPs) and realizes the "all-to-all dispatch by expert
assignment" while building the per-core SPMD input maps; each core runs
expert FFN work in bf16 (fp32 PSUM accumulation); the host applies the
combine weights and scatter-adds results back to [B,N,C].

Load balance: each expert's FFN is split along D_FF into four quarter-units
(exact: gelu is elementwise over F and GEMM2 contracts F, so the four
partial y's just add). The 32 quarter-units are assigned four per core, one
per slot class A-D: slot A holds the two largest experts' quarters, slot B
the next two, etc. Each slot is padded to the max count within its pair, so
per-core padded work is sum over slots of max(pair) — within ~1% of the
perfect-balance floor — instead of 2*max(all counts).
"""

import json
import os
import sys
import types

import numpy as np
import ml_dtypes

D_MODEL = 1024
D_FF = 4096
N_EXPERTS = 8
N_CORES = 8

P = 128
CB = D_MODEL // P      # 8 c-blocks of 128
FQ = D_FF // 4         # F quarter = 1024
FBQ = FQ // P          # 8 f-blocks per quarter
TN = 512               # token tile (matmul moving free dim / one PSUM bank)
SLOTS = ("A", "B", "C", "D")


def _shim_axon_hooks():
    """Register the NTFF profile hook bass_utils looks for under axon; the
    image's `antenv` stub lacks `axon_hooks`."""
    if "antenv.axon_hooks" in sys.modules:
        return
    try:
        import trn_agent_boot.trn_boot as _tb
        hook = _tb._ntff_profile_via_ctypes("/opt/axon/libaxon_pjrt.so")
    except Exception:
        hook = None
    mod = types.ModuleType("antenv.axon_hooks")
    mod.get_axon_ntff_profile_hook = lambda: hook
    mod.set_axon_ntff_profile_hook = lambda h: None
    sys.modules["antenv.axon_hooks"] = mod


_shim_axon_hooks()

import concourse.bass as bass            # noqa: E402
import concourse.tile as tile            # noqa: E402
from concourse import mybir              # noqa: E402
from concourse.bass import ds, ts        # noqa: E402
from concourse.bass_utils import run_bass_kernel_spmd  # noqa: E402


def _fix_multiwait_bir(nc):
    """Split instructions carrying >1 sync wait (the TileContext tail drain)
    into single-wait NoOps; this walrus build rejects multi-wait CTRL
    instructions."""
    raw = bass.Bass.to_json_bytes(nc)
    d = json.loads(raw)
    for f in d["functions"]:
        for b in f["blocks"]:
            out = []
            for i in b["instructions"]:
                si = i.get("sync_info") or {}
                waits = si.get("on_wait") or []
                if len(waits) > 1:
                    for k, w in enumerate(waits[:-1]):
                        out.append({
                            "name": f"{i['name']}_wsplit{k}",
                            "engine": i["engine"],
                            "ins": [], "outs": [],
                            "opcode": "NoOp",
                            "sync_info": {"on_update": [], "on_wait": [w]},
                        })
                    si["on_wait"] = [waits[-1]]
                out.append(i)
            b["instructions"] = out
    fixed = json.dumps(d).encode()
    nc.to_json_bytes = lambda: fixed


_NC_CACHE = {}


def _token_tiles(cap, small_first=False):
    # small_first: a 256-token leading tile halves the bytes the very first
    # matmul waits on; later tiles are prefetched early enough to cover it
    tiles, off = [], 0
    if small_first and cap > TN:
        tiles.append((0, 256))
        off = 256
    while off < cap:
        tw = min(TN, cap - off)
        tiles.append((off, tw))
        off += tw
    return tiles


def _build_moe_kernel(caps):
    """Four quarter-expert FFN units per core (slots A-D), SPMD x8."""
    key = tuple(caps)
    if key in _NC_CACHE:
        return _NC_CACHE[key]

    bf16 = mybir.dt.bfloat16
    f32 = mybir.dt.float32
    Act = mybir.ActivationFunctionType

    nc = bass.Bass("TRN2", target_bir_lowering=False, debug=False,
                   num_devices=N_CORES)

    # all 8 per-slot bias vectors pre-rearranged by the host into one
    # [128, 64] blob: one 128-descriptor DMA instead of 8 DMAs of 1024
    # 4-byte descriptors each (~0.8us queue time + heavy DMA-engine load)
    biasP = nc.declare_dram_parameter("bias", [P, 4 * (FBQ + CB)], f32,
                                      isOutput=False)
    # All DRAM tensors are host-repacked to partition-major layouts so each
    # DMA descriptor covers a full per-partition line (8-16KB) instead of a
    # 1-2KB row segment: the DMA system is descriptor-rate-bound (~80ns per
    # descriptor per engine), not byte-bound, so fat descriptors are what
    # make loads land fast.
    units = []
    for si, (slot, cap) in enumerate(zip(SLOTS, caps)):
        u = {"cap": cap, "slot": slot, "boff": si * (FBQ + CB)}
        u["tiles"] = _token_tiles(cap)
        nt = len(u["tiles"])
        u["nt"] = nt
        # x/y chunk-major: [128, ntiles, 8, 512] -> per-(p,tile) 8KB runs
        u["xT"] = nc.declare_dram_parameter(f"xT{slot}", [P, nt, CB, TN], bf16, isOutput=False)
        # w: [128, 8, 1024] -> per-p 16KB runs
        u["w1t"] = nc.declare_dram_parameter(f"w1t{slot}", [P, CB, FQ], bf16, isOutput=False)
        u["w2t"] = nc.declare_dram_parameter(f"w2t{slot}", [P, FBQ, D_MODEL], bf16, isOutput=False)
        # partials return as bf16: halves the output DMA so total traffic
        # stays under the chip's P0 power-throttle trigger (observed: the
        # f32 version pushed PE from 2.4 to 2.0 GHz); host sums in f32
        u["yT"] = nc.declare_dram_parameter(f"yT{slot}", [P, nt, CB, TN], bf16, isOutput=True)
        u["xr"] = u["xT"].ap()      # [128, nt, 8, 512]
        u["w1r"] = u["w1t"].ap()    # [128, 8, 1024]
        u["w2r"] = u["w2t"].ap()    # [128, 8, 1024]
        u["yr"] = u["yT"].ap()
        units.append(u)

    with tile.TileContext(nc) as tc:
        with (
            tc.tile_pool(name="weights", bufs=1) as wpool,
            tc.tile_pool(name="xin", bufs=3) as xpool,
            tc.tile_pool(name="hbuf", bufs=1) as hpool,
            tc.tile_pool(name="yout", bufs=3) as ypool,
            tc.tile_pool(name="psum", bufs=4, space="PSUM") as psum,
            tc.tile_pool(name="warm", bufs=1) as warmpool,
        ):
            # PE clock warm-up: ~20 dependency-free matmuls on a memset tile
            # run while the first loads are in flight. A cold PE runs its
            # first ~15 matmuls at half clock (427-609ns vs 215ns); burning
            # the ramp on dummies makes the real tile-0 matmuls full-speed.
            wsrc = warmpool.tile([P, P + TN], bf16, tag="wsrc", name="wsrc")
            nc.vector.memset(wsrc[:, :], 0)
            # borrow a rotation slot from the main psum pool's ph tag; the
            # warm matmuls retire long before the 4-deep rotation reuses it
            wps = psum.tile([P, TN], f32, tag="ph", name="wps")
            for _ in range(20):
                nc.tensor.matmul(wps[:, :], lhsT=wsrc[:, 0:P],
                                 rhs=wsrc[:, P:P + TN], start=True, stop=True)
            # ---- loads. SP-ring FIFO order is chosen so PE never waits:
            # tile-0 tokens + first w1 strip first (w1 strip on the ACT ring
            # so it overlaps x0's load), then unit A's remaining weights,
            # then unit A's tile-1 tokens BEFORE units B-D's weight bulk so
            # early tiles stay ahead of the PE.
            # chunked so the first matmul waits on 128KB, not a 1MB monolith:
            # x0 per-k on the sync ring; w1A/w2A chunked along the contract
            # group g (keeps 2KB descriptors — chunking along f/c would 8x
            # the descriptor count). w1A is split across the gpsimd and
            # scalar rings so G1(tile0) is fed at ~2 rings' bandwidth. The
            # scalar queue gets only 4 early dma_starts (issued well before
            # the first gelu needs the engine).
            ua = units[0]
            ua["x0"] = xpool.tile([P, CB, TN], bf16, tag="xt", name="x0A")
            ua["w1_sb"] = wpool.tile([P, CB, FQ], bf16, tag="w1A", name="w1A")
            # early window is aggregate-DMA-ramp-limited (~125GB/s): feed
            # tile-0's consumption order with many small per-g chunks spread
            # across all three rings (empirically the fastest variant): x0
            # per-g on sync, w1A per-g alternating gpsimd/scalar, w2A per-g
            # on gpsimd behind the bias blob.
            bias_sb = wpool.tile([P, 4 * (FBQ + CB)], f32, tag="bias",
                                 name="bias")
            nc.gpsimd.dma_start(bias_sb[:, :], biasP.ap())
            for g in range(CB):
                ring = nc.gpsimd if g % 2 == 0 else nc.scalar
                ring.dma_start(ua["w1_sb"][:, g, :], ua["w1r"][:, g, :])
            for g in range(CB):
                nc.sync.dma_start(ua["x0"][:, g, :], ua["xr"][:, 0, g, :])
            ua["w2_sb"] = wpool.tile([P, FBQ, D_MODEL], bf16, tag="w2A", name="w2A")
            for g in range(FBQ):
                nc.gpsimd.dma_start(ua["w2_sb"][:, g, :], ua["w2r"][:, g, :])

            # unit A tiles 1 and 2 ahead of the remaining weight bulk (the
            # tile-2 load sat behind 12MB of B-D weights on the sync ring and
            # cost an 8.8us PE stall)
            for ti in (1, 2):
                if len(ua["tiles"]) > ti:
                    xt = xpool.tile([P, CB, TN], bf16, tag="xt", name=f"x{ti}A")
                    nc.sync.dma_start(xt[:, :, :], ua["xr"][:, ti, :, :])
                    ua[f"x{ti}"] = xt

            # B-D weight bulk split across the sync and gpsimd rings so it
            # drains ~2x faster and the sync ring frees up for x/y tiles
            for wi, u in enumerate(units[1:]):
                slot = u["slot"]
                u["w1_sb"] = wpool.tile([P, CB, FQ], bf16, tag=f"w1{slot}",
                                        name=f"w1{slot}")
                u["w2_sb"] = wpool.tile([P, FBQ, D_MODEL], bf16, tag=f"w2{slot}",
                                        name=f"w2{slot}")
            for u in units[1:]:
                nc.sync.dma_start(u["w1_sb"][:, :, :], u["w1r"][:, :, :])
                nc.gpsimd.dma_start(u["w2_sb"][:, :, :], u["w2r"][:, :, :])

            # ---- compute: unit A..D tiles in sequence ----
            for u in units:
                for ti, (off, tw) in enumerate(u["tiles"]):
                    if f"x{ti}" in u:
                        xt = u[f"x{ti}"]
                    else:
                        xt = xpool.tile([P, CB, TN], bf16, tag="xt")
                        nc.sync.dma_start(xt[:, :, :], u["xr"][:, ti, :, :])

                    ht = hpool.tile([P, FBQ, TN], bf16, tag="ht")
                    for m in range(FBQ):
                        ph = psum.tile([P, TN], f32, tag="ph")
                        for k in range(CB):
                            nc.tensor.matmul(
                                ph[:, :tw],
                                lhsT=u["w1_sb"][:, k, ts(m, P)],
                                rhs=xt[:, k, :tw],
                                start=(k == 0), stop=(k == CB - 1),
                            )
                        nc.scalar.activation(ht[:, m, :tw], ph[:, :tw], Act.Gelu,
                                             bias=bias_sb[:, u["boff"] + m:u["boff"] + m + 1])

                    last = (u is units[-1]) and (ti == len(u["tiles"]) - 1)
                    yt = ypool.tile([P, CB, TN], bf16, tag="yt")
                    for c in range(CB):
                        py = psum.tile([P, TN], f32, tag="py")
                        for k in range(FBQ):
                            nc.tensor.matmul(
                                py[:, :tw],
                                lhsT=u["w2_sb"][:, k, ts(c, P)],
                                rhs=ht[:, k, :tw],
                                start=(k == 0), stop=(k == FBQ - 1),
                            )
                        bo = u["boff"] + FBQ
                        nc.scalar.add(yt[:, c, :tw], py[:, :tw],
                                      bias_sb[:, bo + c:bo + c + 1])
                        if last:
                            # final tile: per-block stores overlap the tail
                            # GEMM2 blocks instead of one post-loop DMA
                            nc.sync.dma_start(u["yr"][:, ti, c, :tw],
                                              yt[:, c, :tw])
                    if not last:
                        # full-chunk store (8KB descriptors); the pad zone
                        # carries stale data the host never reads
                        nc.sync.dma_start(u["yr"][:, ti, :, :], yt[:, :, :])

    _fix_multiwait_bir(nc)
    _NC_CACHE[key] = nc
    return nc


def _route(xf, router_w, k):
    """Replicate the reference router numerics (f32 softmax, top-k, renorm)."""
    logits = xf @ router_w.T.astype(np.float32)          # [T, E]
    m = logits.max(axis=-1, keepdims=True)
    e = np.exp(logits - m, dtype=np.float32)
    probs = e / e.sum(axis=-1, keepdims=True)
    # descending, ties -> lower index first (matches jax.lax.top_k)
    idx = np.argsort(-probs, axis=-1, kind="stable")[:, :k]   # [T, k]
    w = np.take_along_axis(probs, idx, axis=-1)               # [T, k]
    w = w / (w.sum(axis=-1, keepdims=True) + 1e-9)
    return idx, w


def _align16(n):
    # 4-token (8-byte) alignment keeps DMA rows aligned; finer than 16
    # saves ~20 padded tokens across the four slots
    return max(P, -(-n // 4) * 4)


def kernel(x, router_w, expert_w1, expert_b1, expert_w2, expert_b2, top_k):
    x = np.asarray(x)
    router_w = np.asarray(router_w, dtype=np.float32)
    expert_w1 = np.asarray(expert_w1, dtype=np.float32)
    expert_b1 = np.asarray(expert_b1, dtype=np.float32)
    expert_w2 = np.asarray(expert_w2, dtype=np.float32)
    expert_b2 = np.asarray(expert_b2, dtype=np.float32)
    k = int(np.asarray(top_k))
    Bq, Nq, C = x.shape
    Tq = Bq * Nq
    E = expert_w1.shape[0]
    xf = np.ascontiguousarray(x.reshape(Tq, C), dtype=np.float32)

    idx, w = _route(xf, router_w, k)

    tok_idx, tok_w = [], []
    for e in range(E):
        mask = idx == e
        sel = np.nonzero(mask.any(axis=-1))[0]
        tok_idx.append(sel)
        tok_w.append((w * mask).sum(axis=-1)[sel].astype(np.float32))
    counts = np.array([len(s) for s in tok_idx])

    # slot s holds the quarters of the experts ranked 2s and 2s+1 by count;
    # cores 0-3 take quarters 0-3 of the first, cores 4-7 of the second
    order = np.argsort(-counts, kind="stable")
    caps = [_align16(int(counts[order[2 * s]])) for s in range(4)]

    nc = _build_moe_kernel(caps)

    # one xT per expert, shared by its four quarter-units; chunk-major
    # [128, ntiles, 8, 512] so every tile load is 128 8KB descriptors
    xTs, slot_of = {}, {}
    for s in range(4):
        nt = (caps[s] + TN - 1) // TN
        for j in (0, 1):
            e = int(order[2 * s + j])
            slot_of[e] = s
            tmp = np.zeros((nt * TN, C), dtype=np.float32)
            tmp[:counts[e]] = xf[tok_idx[e]]
            xTs[e] = np.ascontiguousarray(
                tmp.reshape(nt, TN, CB, P).transpose(3, 0, 2, 1)
            ).astype(ml_dtypes.bfloat16)

    in_maps = [dict() for _ in range(N_CORES)]
    placement = {}          # (expert, quarter) -> (core, slot name)
    FBQ_, CB_ = FQ // P, C // P
    blobs = [np.zeros((P, 4 * (FBQ_ + CB_)), dtype=np.float32)
             for _ in range(N_CORES)]
    for s, slot in enumerate(SLOTS):
        for core in range(N_CORES):
            e = int(order[2 * s + (0 if core < 4 else 1)])
            q = core % 4
            placement[(e, q)] = (core, slot)
            f0, f1 = q * FQ, (q + 1) * FQ
            b2 = expert_b2[e] if q == 0 else np.zeros(C, dtype=np.float32)
            bo = s * (FBQ_ + CB_)
            blobs[core][:, bo:bo + FBQ_] = expert_b1[e, f0:f1].reshape(FBQ_, P).T
            blobs[core][:, bo + FBQ_:bo + FBQ_ + CB_] = b2.reshape(CB_, P).T
            w1q = expert_w1[e, f0:f1].T.reshape(CB_, P, FQ).transpose(1, 0, 2)
            w2q = expert_w2[e, :, f0:f1].T.reshape(FBQ_, P, C).transpose(1, 0, 2)
            in_maps[core].update({
                f"xT{slot}": xTs[e],
                f"w1t{slot}": np.ascontiguousarray(w1q).astype(ml_dtypes.bfloat16),
                f"w2t{slot}": np.ascontiguousarray(w2q).astype(ml_dtypes.bfloat16),
            })
    for core in range(N_CORES):
        in_maps[core]["bias"] = blobs[core]

    trace = os.environ.get("BASS_MOE_TRACE") == "1"
    res = run_bass_kernel_spmd(
        nc, in_maps, core_ids=list(range(N_CORES)),
        trace=trace,
        tmpdir=os.environ.get("BASS_MOE_TMPDIR") if trace else None,
    )
    if trace:
        kernel.last_exec_time_ns = res.exec_time_ns
        kernel.last_trace = (res.instructions_and_trace or (None, None))[1]

    out = np.zeros((Tq, C), dtype=np.float32)
    for e in range(E):
        cnt = counts[e]
        if not cnt:
            continue
        acc = np.zeros((cnt, C), dtype=np.float32)
        for q in range(4):
            core, slot = placement[(e, q)]
            yq = res.results[core][f"yT{slot}"]        # [128, nt, 8, 512]
            nt = yq.shape[1]
            yq = yq.transpose(1, 3, 2, 0).reshape(nt * TN, C)[:cnt]
            acc += yq.astype(np.float32)
        out[tok_idx[e]] += acc * tok_w[e][:, None]
    return out.reshape(Bq, Nq, C).astype(x.dtype)



# revision 33
# speedup vs baseline: 1.0259x; 1.0029x over previous
"""MoE layer (top-k routing) on 8 Trainium2 NeuronCores.

Expert-parallel per the sharding hint: the host computes router softmax +
top-k (0.1% of FLOPs) and realizes the "all-to-all dispatch by expert
assignment" while building the per-core SPMD input maps; each core runs
expert FFN work in bf16 (fp32 PSUM accumulation); the host applies the
combine weights and scatter-adds results back to [B,N,C].

Load balance: each expert's FFN is split along D_FF into four quarter-units
(exact: gelu is elementwise over F and GEMM2 contracts F, so the four
partial y's just add). The 32 quarter-units are assigned four per core, one
per slot class A-D: slot A holds the two largest experts' quarters, slot B
the next two, etc. Each slot is padded to the max count within its pair, so
per-core padded work is sum over slots of max(pair) — within ~1% of the
perfect-balance floor — instead of 2*max(all counts).
"""

import json
import os
import sys
import types

import numpy as np
import ml_dtypes

D_MODEL = 1024
D_FF = 4096
N_EXPERTS = 8
N_CORES = 8

P = 128
CB = D_MODEL // P      # 8 c-blocks of 128
FQ = D_FF // 4         # F quarter = 1024
FBQ = FQ // P          # 8 f-blocks per quarter
TN = 1024              # token chunk (bf16 matmul moving max; psum = 2 banks)
SLOTS = ("A", "B", "C", "D")


def _shim_axon_hooks():
    """Register the NTFF profile hook bass_utils looks for under axon; the
    image's `antenv` stub lacks `axon_hooks`."""
    if "antenv.axon_hooks" in sys.modules:
        return
    try:
        import trn_agent_boot.trn_boot as _tb
        hook = _tb._ntff_profile_via_ctypes("/opt/axon/libaxon_pjrt.so")
    except Exception:
        hook = None
    mod = types.ModuleType("antenv.axon_hooks")
    mod.get_axon_ntff_profile_hook = lambda: hook
    mod.set_axon_ntff_profile_hook = lambda h: None
    sys.modules["antenv.axon_hooks"] = mod


_shim_axon_hooks()

import concourse.bass as bass            # noqa: E402
import concourse.tile as tile            # noqa: E402
from concourse import mybir              # noqa: E402
from concourse.bass import ds, ts        # noqa: E402
from concourse.bass_utils import run_bass_kernel_spmd  # noqa: E402


def _fix_multiwait_bir(nc):
    """Split instructions carrying >1 sync wait (the TileContext tail drain)
    into single-wait NoOps; this walrus build rejects multi-wait CTRL
    instructions."""
    raw = bass.Bass.to_json_bytes(nc)
    d = json.loads(raw)
    for f in d["functions"]:
        for b in f["blocks"]:
            out = []
            for i in b["instructions"]:
                si = i.get("sync_info") or {}
                waits = si.get("on_wait") or []
                if len(waits) > 1:
                    for k, w in enumerate(waits[:-1]):
                        out.append({
                            "name": f"{i['name']}_wsplit{k}",
                            "engine": i["engine"],
                            "ins": [], "outs": [],
                            "opcode": "NoOp",
                            "sync_info": {"on_update": [], "on_wait": [w]},
                        })
                    si["on_wait"] = [waits[-1]]
                out.append(i)
            b["instructions"] = out
    fixed = json.dumps(d).encode()
    nc.to_json_bytes = lambda: fixed


_NC_CACHE = {}


def _token_tiles(cap):
    """(chunk_idx, offset_in_chunk, width) triples; chunk 0 is split into
    two 512-token tiles so the first matmul group waits on half the bytes
    and the startup window stays DMA-light."""
    tiles = []
    nt = -(-cap // TN)
    for ci in range(nt):
        cw = min(TN, cap - ci * TN)
        if ci == 0 and cw == TN:
            tiles.append((0, 0, TN // 2))
            tiles.append((0, TN // 2, TN // 2))
        else:
            tiles.append((ci, 0, cw))
    return tiles


def _build_moe_kernel(caps):
    """Four quarter-expert FFN units per core (slots A-D), SPMD x8."""
    key = tuple(caps)
    if key in _NC_CACHE:
        return _NC_CACHE[key]

    bf16 = mybir.dt.bfloat16
    f32 = mybir.dt.float32
    Act = mybir.ActivationFunctionType

    nc = bass.Bass("TRN2", target_bir_lowering=False, debug=False,
                   num_devices=N_CORES)

    # all 8 per-slot bias vectors pre-rearranged by the host into one
    # [128, 64] blob: one 128-descriptor DMA instead of 8 DMAs of 1024
    # 4-byte descriptors each (~0.8us queue time + heavy DMA-engine load)
    biasP = nc.declare_dram_parameter("bias", [P, 4 * (FBQ + CB)], f32,
                                      isOutput=False)
    # All DRAM tensors are host-repacked to partition-major layouts so each
    # DMA descriptor covers a full per-partition line (8-16KB) instead of a
    # 1-2KB row segment: the DMA system is descriptor-rate-bound (~80ns per
    # descriptor per engine), not byte-bound, so fat descriptors are what
    # make loads land fast.
    units = []
    for si, (slot, cap) in enumerate(zip(SLOTS, caps)):
        u = {"cap": cap, "slot": slot, "boff": si * (FBQ + CB)}
        u["tiles"] = _token_tiles(cap)
        nt = -(-cap // TN)
        u["nt"] = nt
        # x/y chunk-major: [128, ntiles, 8, 512] -> per-(p,tile) 8KB runs
        u["xT"] = nc.declare_dram_parameter(f"xT{slot}", [P, nt, CB, TN], bf16, isOutput=False)
        # w: [128, 8, 1024] -> per-p 16KB runs
        u["w1t"] = nc.declare_dram_parameter(f"w1t{slot}", [P, CB, FQ], bf16, isOutput=False)
        u["w2t"] = nc.declare_dram_parameter(f"w2t{slot}", [P, FBQ, D_MODEL], bf16, isOutput=False)
        # partials return as bf16: halves the output DMA so total traffic
        # stays under the chip's P0 power-throttle trigger (observed: the
        # f32 version pushed PE from 2.4 to 2.0 GHz); host sums in f32
        u["yT"] = nc.declare_dram_parameter(f"yT{slot}", [P, nt, CB, TN], bf16, isOutput=True)
        u["xr"] = u["xT"].ap()      # [128, nt, 8, 512]
        u["w1r"] = u["w1t"].ap()    # [128, 8, 1024]
        u["w2r"] = u["w2t"].ap()    # [128, 8, 1024]
        u["yr"] = u["yT"].ap()
        units.append(u)

    with tile.TileContext(nc) as tc:
        with (
            tc.tile_pool(name="weights", bufs=1) as wpool,
            tc.tile_pool(name="xin", bufs=3) as xpool,
            tc.tile_pool(name="hbuf", bufs=1) as hpool,
            tc.tile_pool(name="yout", bufs=2) as ypool,
            tc.tile_pool(name="psum", bufs=2, space="PSUM") as psum,
            tc.tile_pool(name="warm", bufs=1) as warmpool,
        ):
            # PE clock warm-up: ~20 dependency-free matmuls on a memset tile
            # run while the first loads are in flight. A cold PE runs its
            # first ~15 matmuls at half clock (427-609ns vs 215ns); burning
            # the ramp on dummies makes the real tile-0 matmuls full-speed.
            wsrc = warmpool.tile([P, P + 256], bf16, tag="wsrc", name="wsrc")
            nc.vector.memset(wsrc[:, :], 0)
            # borrow a rotation slot from the main psum pool's ph tag; the
            # warm matmuls retire long before the rotation reuses it
            wps = psum.tile([P, TN], f32, tag="ph", name="wps")
            for _ in range(28):
                nc.tensor.matmul(wps[:, 0:256], lhsT=wsrc[:, 0:P],
                                 rhs=wsrc[:, P:P + 256], start=True, stop=True)
            # ---- loads. SP-ring FIFO order is chosen so PE never waits:
            # tile-0 tokens + first w1 strip first (w1 strip on the ACT ring
            # so it overlaps x0's load), then unit A's remaining weights,
            # then unit A's tile-1 tokens BEFORE units B-D's weight bulk so
            # early tiles stay ahead of the PE.
            # chunked so the first matmul waits on 128KB, not a 1MB monolith:
            # x0 per-k on the sync ring; w1A/w2A chunked along the contract
            # group g (keeps 2KB descriptors — chunking along f/c would 8x
            # the descriptor count). w1A is split across the gpsimd and
            # scalar rings so G1(tile0) is fed at ~2 rings' bandwidth. The
            # scalar queue gets only 4 early dma_starts (issued well before
            # the first gelu needs the engine).
            ua = units[0]
            # early window is aggregate-DMA-ramp-limited (~125GB/s): feed
            # tile-0's consumption order with small per-g chunks spread
            # across all three rings: x chunk-0 first half per-g on sync,
            # w1A per-g split gpsimd/scalar, w2A per-g on gpsimd.
            ua["xc0"] = xpool.tile([P, CB, TN], bf16, tag="xt", name="x0A")
            ua["w1_sb"] = wpool.tile([P, CB, FQ], bf16, tag="w1A", name="w1A")
            bias_sb = wpool.tile([P, 4 * (FBQ + CB)], f32, tag="bias",
                                 name="bias")
            nc.gpsimd.dma_start(bias_sb[:, :], biasP.ap())
            H = TN // 2
            for g in range(CB):
                ring = nc.gpsimd if g < 4 else nc.scalar
                ring.dma_start(ua["w1_sb"][:, g, :], ua["w1r"][:, g, :])
            for g in range(CB):
                nc.sync.dma_start(ua["xc0"][:, g, 0:H], ua["xr"][:, 0, g, 0:H])
            nc.sync.dma_start(ua["xc0"][:, :, H:TN], ua["xr"][:, 0, :, H:TN])
            ua["w2_sb"] = wpool.tile([P, FBQ, D_MODEL], bf16, tag="w2A", name="w2A")
            for g in range(FBQ):
                nc.gpsimd.dma_start(ua["w2_sb"][:, g, :], ua["w2r"][:, g, :])

            # unit A chunks 1-2 ahead of the remaining weight bulk
            for ci in (1, 2):
                if ci < ua["nt"]:
                    xt = xpool.tile([P, CB, TN], bf16, tag="xt", name=f"x{ci}A")
                    nc.sync.dma_start(xt[:, :, :], ua["xr"][:, ci, :, :])
                    ua[f"xc{ci}"] = xt

            # B/C weights as fat monoliths split over sync+gpsimd. Unit D
            # REUSES unit A's w1 and unit B's w2 SBUF buffers (tag sharing):
            # its loads sit on the otherwise-idle gpsimd queue and wait for
            # the donor unit's last read, saving 32KB/partition of SBUF.
            for u in units[1:3]:
                slot = u["slot"]
                u["w1_sb"] = wpool.tile([P, CB, FQ], bf16, tag=f"w1{slot}",
                                        name=f"w1{slot}")
                u["w2_sb"] = wpool.tile([P, FBQ, D_MODEL], bf16, tag=f"w2{slot}",
                                        name=f"w2{slot}")
            ud = units[3]
            ud["w1_sb"] = wpool.tile([P, CB, FQ], bf16, tag="w1A", name="w1D")
            ud["w2_sb"] = wpool.tile([P, FBQ, D_MODEL], bf16, tag="w2B",
                                     name="w2D")
            for u in units[1:3]:
                nc.sync.dma_start(u["w1_sb"][:, :, :], u["w1r"][:, :, :])
                nc.gpsimd.dma_start(u["w2_sb"][:, :, :], u["w2r"][:, :, :])
            nc.gpsimd.dma_start(ud["w1_sb"][:, :, :], ud["w1r"][:, :, :])
            nc.gpsimd.dma_start(ud["w2_sb"][:, :, :], ud["w2r"][:, :, :])

            # ---- compute: unit A..D tiles in sequence ----
            for u in units:
                cur_ci = None
                for ti, (ci, off, tw) in enumerate(u["tiles"]):
                    if ci != cur_ci:
                        if f"xc{ci}" in u:
                            xt = u[f"xc{ci}"]
                        else:
                            xt = xpool.tile([P, CB, TN], bf16, tag="xt")
                            nc.sync.dma_start(xt[:, :, :], u["xr"][:, ci, :, :])
                        cur_ci = ci
                        yt = ypool.tile([P, CB, TN], bf16, tag="yt")

                    ht = hpool.tile([P, FBQ, TN], bf16, tag="ht")
                    for m in range(FBQ):
                        ph = psum.tile([P, TN], f32, tag="ph")
                        for k in range(CB):
                            nc.tensor.matmul(
                                ph[:, :tw],
                                lhsT=u["w1_sb"][:, k, ts(m, P)],
                                rhs=xt[:, k, off:off + tw],
                                start=(k == 0), stop=(k == CB - 1),
                            )
                        nc.scalar.activation(ht[:, m, :tw], ph[:, :tw], Act.Gelu,
                                             bias=bias_sb[:, u["boff"] + m:u["boff"] + m + 1])

                    last = (u is units[-1]) and (ti == len(u["tiles"]) - 1)
                    chunk_done = (ti == len(u["tiles"]) - 1) or \
                        (u["tiles"][ti + 1][0] != ci)
                    for c in range(CB):
                        py = psum.tile([P, TN], f32, tag="py")
                        for k in range(FBQ):
                            nc.tensor.matmul(
                                py[:, :tw],
                                lhsT=u["w2_sb"][:, k, ts(c, P)],
                                rhs=ht[:, k, :tw],
                                start=(k == 0), stop=(k == FBQ - 1),
                            )
                        bo = u["boff"] + FBQ
                        nc.scalar.add(yt[:, c, off:off + tw], py[:, :tw],
                                      bias_sb[:, bo + c:bo + c + 1])
                        if last:
                            # final tile: per-block stores overlap the tail
                            # GEMM2 blocks instead of one post-loop DMA
                            nc.sync.dma_start(u["yr"][:, ci, c, off:off + tw],
                                              yt[:, c, off:off + tw])
                    if chunk_done and not last:
                        # full-chunk store (16KB descriptors); pad zones
                        # carry stale data the host never reads
                        nc.sync.dma_start(u["yr"][:, ci, :, :], yt[:, :, :])

    _fix_multiwait_bir(nc)
    _NC_CACHE[key] = nc
    return nc


def _route(xf, router_w, k):
    """Replicate the reference router numerics (f32 softmax, top-k, renorm)."""
    logits = xf @ router_w.T.astype(np.float32)          # [T, E]
    m = logits.max(axis=-1, keepdims=True)
    e = np.exp(logits - m, dtype=np.float32)
    probs = e / e.sum(axis=-1, keepdims=True)
    # descending, ties -> lower index first (matches jax.lax.top_k)
    idx = np.argsort(-probs, axis=-1, kind="stable")[:, :k]   # [T, k]
    w = np.take_along_axis(probs, idx, axis=-1)               # [T, k]
    w = w / (w.sum(axis=-1, keepdims=True) + 1e-9)
    return idx, w


def _align16(n):
    # 4-token (8-byte) alignment keeps DMA rows aligned; finer than 16
    # saves ~20 padded tokens across the four slots
    return max(P, -(-n // 4) * 4)


def kernel(x, router_w, expert_w1, expert_b1, expert_w2, expert_b2, top_k):
    x = np.asarray(x)
    router_w = np.asarray(router_w, dtype=np.float32)
    expert_w1 = np.asarray(expert_w1, dtype=np.float32)
    expert_b1 = np.asarray(expert_b1, dtype=np.float32)
    expert_w2 = np.asarray(expert_w2, dtype=np.float32)
    expert_b2 = np.asarray(expert_b2, dtype=np.float32)
    k = int(np.asarray(top_k))
    Bq, Nq, C = x.shape
    Tq = Bq * Nq
    E = expert_w1.shape[0]
    xf = np.ascontiguousarray(x.reshape(Tq, C), dtype=np.float32)

    idx, w = _route(xf, router_w, k)

    tok_idx, tok_w = [], []
    for e in range(E):
        mask = idx == e
        sel = np.nonzero(mask.any(axis=-1))[0]
        tok_idx.append(sel)
        tok_w.append((w * mask).sum(axis=-1)[sel].astype(np.float32))
    counts = np.array([len(s) for s in tok_idx])

    # slot s holds the quarters of the experts ranked 2s and 2s+1 by count;
    # cores 0-3 take quarters 0-3 of the first, cores 4-7 of the second
    order = np.argsort(-counts, kind="stable")
    caps = [_align16(int(counts[order[2 * s]])) for s in range(4)]

    nc = _build_moe_kernel(caps)

    # one xT per expert, shared by its four quarter-units; chunk-major
    # [128, ntiles, 8, 512] so every tile load is 128 8KB descriptors
    xTs, slot_of = {}, {}
    for s in range(4):
        nt = (caps[s] + TN - 1) // TN
        for j in (0, 1):
            e = int(order[2 * s + j])
            slot_of[e] = s
            tmp = np.zeros((nt * TN, C), dtype=np.float32)
            tmp[:counts[e]] = xf[tok_idx[e]]
            xTs[e] = np.ascontiguousarray(
                tmp.reshape(nt, TN, CB, P).transpose(3, 0, 2, 1)
            ).astype(ml_dtypes.bfloat16)

    in_maps = [dict() for _ in range(N_CORES)]
    placement = {}          # (expert, quarter) -> (core, slot name)
    FBQ_, CB_ = FQ // P, C // P
    blobs = [np.zeros((P, 4 * (FBQ_ + CB_)), dtype=np.float32)
             for _ in range(N_CORES)]
    for s, slot in enumerate(SLOTS):
        for core in range(N_CORES):
            e = int(order[2 * s + (0 if core < 4 else 1)])
            q = core % 4
            placement[(e, q)] = (core, slot)
            f0, f1 = q * FQ, (q + 1) * FQ
            b2 = expert_b2[e] if q == 0 else np.zeros(C, dtype=np.float32)
            bo = s * (FBQ_ + CB_)
            blobs[core][:, bo:bo + FBQ_] = expert_b1[e, f0:f1].reshape(FBQ_, P).T
            blobs[core][:, bo + FBQ_:bo + FBQ_ + CB_] = b2.reshape(CB_, P).T
            w1q = expert_w1[e, f0:f1].T.reshape(CB_, P, FQ).transpose(1, 0, 2)
            w2q = expert_w2[e, :, f0:f1].T.reshape(FBQ_, P, C).transpose(1, 0, 2)
            in_maps[core].update({
                f"xT{slot}": xTs[e],
                f"w1t{slot}": np.ascontiguousarray(w1q).astype(ml_dtypes.bfloat16),
                f"w2t{slot}": np.ascontiguousarray(w2q).astype(ml_dtypes.bfloat16),
            })
    for core in range(N_CORES):
        in_maps[core]["bias"] = blobs[core]

    trace = os.environ.get("BASS_MOE_TRACE") == "1"
    res = run_bass_kernel_spmd(
        nc, in_maps, core_ids=list(range(N_CORES)),
        trace=trace,
        tmpdir=os.environ.get("BASS_MOE_TMPDIR") if trace else None,
    )
    if trace:
        kernel.last_exec_time_ns = res.exec_time_ns
        kernel.last_trace = (res.instructions_and_trace or (None, None))[1]

    out = np.zeros((Tq, C), dtype=np.float32)
    for e in range(E):
        cnt = counts[e]
        if not cnt:
            continue
        acc = np.zeros((cnt, C), dtype=np.float32)
        for q in range(4):
            core, slot = placement[(e, q)]
            yq = res.results[core][f"yT{slot}"]        # [128, nt, 8, 512]
            nt = yq.shape[1]
            yq = yq.transpose(1, 3, 2, 0).reshape(nt * TN, C)[:cnt]
            acc += yq.astype(np.float32)
        out[tok_idx[e]] += acc * tok_w[e][:, None]
    return out.reshape(Bq, Nq, C).astype(x.dtype)

